# revision 29
# baseline (speedup 1.0000x reference)
"""Trainium2 Bass kernel for the BsPINN Helmholtz loss (nn_BsPINN_45938970198305).

Forward-Laplacian propagation with fp8(e4m3) DoubleRow matmuls:
  streams per hidden layer l (all fp8 in SBUF, fp32 in PSUM):
    v8 = sin(z)                     gx8, gy8 = cos(z) * (zx, zy)
    m18 = cos(z) * zt               q8 = v * (zx^2 + zy^2)
  next layer (fp8 DoubleRow, K=256 per matmul):
    z' = W^T v8;  zx' = W^T gx8;  zy' = W^T gy8;  zt' = W^T m18 + W^T q8
  Layer-0 tangent constants folded into pre-scaled W1 copies (W1x/W1y/W1q);
  layer-0 emits sin/cos directly as fp8 (cos stays near 1; quantization of it
  is within the fp8 noise floor of the streams - verified numerically).
  Final: E = -W5^T(m1+q) + k0^2 W5^T v + (f + k0^2 b5), loss via E^2 accums.
  Boundary points: plain forward sin-pass, E_b = W5^T v + b5.

Sharding: data-parallel over points; 8 cores x (8192 domain + 2048 boundary);
weights replicated. Host combines 20 partial sums of squares per core.
"""

import numpy as np
import ml_dtypes

import concourse.bass as bass
import concourse.bacc as bacc_mod
import concourse.mybir as mybir
import concourse.tile as tile
from concourse.bass_utils import run_bass_kernel_spmd

bf16 = ml_dtypes.bfloat16
f8 = ml_dtypes.float8_e4m3
FP32 = mybir.dt.float32
BF16 = mybir.dt.bfloat16
F8 = mybir.dt.float8e4
AF = mybir.ActivationFunctionType
ALU = mybir.AluOpType
PM = mybir.MatmulPerfMode

NCORES = 8
ND, NB = 65536, 16384
TDOM, TBND = ND // NCORES, NB // NCORES  # 8192, 2048 points per core
T = 512                                  # points per tile
NTD, NTB = TDOM // T, TBND // T          # 16, 4
K0 = 8.0
K0SQ = K0 * K0
PI_2 = float(np.pi / 2)

# engine knobs (tuned against TimelineSim): where each elementwise op runs
SQ_ENG = {1: "A", 2: "D", 3: "P", 4: "A"}   # Square(pxy) per layer
Q_ENG = {1: "P", 2: "P", 3: "P", 4: "P"}    # q = v*r2 per layer
R2_ENG = {1: "D", 2: "D", 3: "D", 4: "D"}   # r2 = sqx+sqy per layer


def _bcast(tile_ap, n):
    """AP reading tile_ap (a [128, T] slice) broadcast n times on a middle dim."""
    return bass.AP(tile_ap.tensor, tile_ap.offset,
                   [tile_ap.ap[0], [0, n], tile_ap.ap[1]])


def build_nc(ntd=NTD, ntb=NTB):
    from contextlib import ExitStack

    td, tb = ntd * T, ntb * T
    nc = bacc_mod.Bacc("TRN2", target_bir_lowering=False)

    xa_d = nc.dram_tensor("xa", [2, td], BF16, kind="ExternalInput")
    xb_d = nc.dram_tensor("xb", [2, tb], BF16, kind="ExternalInput")
    fb_d = nc.dram_tensor("fb", [1, td], BF16, kind="ExternalInput")
    bb_d = nc.dram_tensor("bb", [1, tb], BF16, kind="ExternalInput")
    w0_d = nc.dram_tensor("w0", [2, 512], BF16, kind="ExternalInput")
    w_d = {
        l: nc.dram_tensor(f"w{l}", [128, 4, 512], F8, kind="ExternalInput")
        for l in (1, 2, 3, 4)
    }
    wf_d = {
        s: nc.dram_tensor(f"w1{s}", [128, 4, 512], F8, kind="ExternalInput")
        for s in ("x", "y", "q")
    }
    w5_d = nc.dram_tensor("w5", [128, 4, 16], F8, kind="ExternalInput")
    w16_d = {
        l: nc.dram_tensor(f"w16_{l}", [128, 4, 512], BF16, kind="ExternalInput")
        for l in (1, 2, 3, 4)
    }
    w516_d = nc.dram_tensor("w516", [128, 4, 3], BF16, kind="ExternalInput")
    bias_d = nc.dram_tensor("bias", [128, 5, 4, 2], FP32, kind="ExternalInput")
    out_d = nc.dram_tensor("out", [1, 32], FP32, kind="ExternalOutput")

    with tile.TileContext(nc) as tc, ExitStack() as ctx:
        singles = ctx.enter_context(tc.tile_pool(name="singles", bufs=1))
        acts = ctx.enter_context(tc.tile_pool(name="acts", bufs=3))
        ew = ctx.enter_context(tc.tile_pool(name="ew", bufs=3))
        pp = ctx.enter_context(tc.tile_pool(name="pp", bufs=2, space="PSUM"))

        # DMA order matters at startup: tile 0 needs bias (ACT warmup), w0,
        # xa chunk 0, then the L1 weight family; later layers' weights follow.
        bias_sb = singles.tile([128, 5, 4, 2], FP32, name="bias_sb")
        nc.sync.dma_start(out=bias_sb, in_=bias_d[:])
        w0_sb = singles.tile([2, 512], BF16, name="w0_sb")
        nc.sync.dma_start(out=w0_sb, in_=w0_d[:])
        xa_sb = singles.tile([2, td], BF16, name="xa_sb")
        fb_sb = singles.tile([1, td], BF16, name="fb_sb")
        s4 = slice(0, td // 4)
        nc.sync.dma_start(out=xa_sb[:, s4], in_=xa_d[:, s4])
        wf_sb = {}
        for s in ("x", "y", "q"):
            wf_sb[s] = singles.tile([128, 4, 512], F8, name=f"w1{s}_sb", tag=f"w1{s}_sb")
            nc.sync.dma_start(out=wf_sb[s], in_=wf_d[s][:])
        w_sb = {}
        w16_sb = {}
        for l in (1, 2, 3, 4):
            w_sb[l] = singles.tile([128, 4, 512], F8, name=f"w{l}_sb", tag=f"w{l}_sb")
            w16_sb[l] = singles.tile([128, 4, 512], BF16, name=f"w16_{l}_sb", tag=f"w16_{l}_sb")
        for l in (1, 2, 3, 4):
            nc.sync.dma_start(out=w_sb[l], in_=w_d[l][:])
            nc.sync.dma_start(out=w16_sb[l], in_=w16_d[l][:])
        for c4 in range(1, 4):
            s4 = slice(c4 * td // 4, (c4 + 1) * td // 4)
            nc.sync.dma_start(out=xa_sb[:, s4], in_=xa_d[:, s4])
        for c4 in range(4):
            s4 = slice(c4 * td // 4, (c4 + 1) * td // 4)
            nc.sync.dma_start(out=fb_sb[:, s4], in_=fb_d[:, s4])
        xb_sb = singles.tile([2, tb], BF16, name="xb_sb")
        nc.sync.dma_start(out=xb_sb, in_=xb_d[:])
        bb_sb = singles.tile([1, tb], BF16, name="bb_sb")
        nc.sync.dma_start(out=bb_sb, in_=bb_d[:])
        w5_sb = singles.tile([128, 4, 16], F8, name="w5_sb")
        nc.sync.dma_start(out=w5_sb, in_=w5_d[:])
        w516_sb = singles.tile([128, 4, 3], BF16, name="w516_sb")
        nc.sync.dma_start(out=w516_sb, in_=w516_d[:])

        out_sb = singles.tile([1, 32], FP32, name="out_sb")
        nc.vector.memset(out_sb, 0.0)
        one_sb = singles.tile([1, 1], BF16, name="one_sb")
        nc.vector.memset(one_sb, 1.0)

        # Warmup activation: absorbs the one-time ACT table load and bias-DMA
        # wait so later ACTIVATEs carry at most 2 sync waits.
        warm_sb = singles.tile([1, 1], FP32, name="warm_sb")
        nc.scalar.activation(warm_sb, bias_sb[0:1, 0, 0, 0:1], AF.Sin)

        def dr(out, wtile, jpair, msl_or_col, rhs, start, stop):
            nc.tensor.matmul(
                out, wtile[:, 2 * jpair : 2 * jpair + 2, msl_or_col], rhs,
                start=start, stop=stop, perf_mode=PM.DoubleRow,
            )

        # ---------------- tiles (boundary interleaved every 4th) ----------------
        def domain_chunks(ti):
            csl = slice(ti * T, (ti + 1) * T)

            # layer 0: z0 = W0^T xa (K=2, bf16); sin/cos pairs -> fp8
            v8 = acts.tile([128, 4, T], F8, name=f"v80_{ti}", tag="v80")
            c08 = acts.tile([128, 4, T], F8, name=f"c08_{ti}", tag="c08")
            for p in range(2):
                pq0 = pp.tile([128, 3, T], FP32, name=f"pq0_{ti}_{p}", tag="pxyt")
                for mm_ in range(2):
                    nc.tensor.matmul(
                        pq0[:, mm_, :],
                        w0_sb[:, (2 * p + mm_) * 128 : (2 * p + mm_ + 1) * 128],
                        xa_sb[:, csl], start=True, stop=True,
                    )
                nc.scalar.activation(v8[:, 2 * p : 2 * p + 2, :], pq0[:, 0:2, :],
                                     AF.Sin, bias=bias_sb[:, 0, 2 * p, 0:1])
                if C0_ENG == "A":
                    nc.scalar.activation(c08[:, 2 * p : 2 * p + 2, :], pq0[:, 0:2, :],
                                         AF.Sin, bias=bias_sb[:, 0, 2 * p, 1:2])
                else:
                    s08 = ew.tile([128, 2, T], BF16, name=f"s08_{ti}_{p}",
                                  tag="s08", bufs=2)
                    nc.vector.tensor_tensor(s08, v8[:, 2 * p : 2 * p + 2, :],
                                            v8[:, 2 * p : 2 * p + 2, :], ALU.mult)
                    nc.vector.tensor_scalar(c08[:, 2 * p : 2 * p + 2, :], s08,
                                            -0.5, 1.0, op0=ALU.mult, op1=ALU.add)
            yield

            # hidden layers 1..4: psum per m = [z | x | y | t]
            gm = None
            for l in range(1, 5):
                v8_n = acts.tile([128, 4, T], BF16, name=f"v8_{l}_{ti}", tag="v8")
                ct_n = acts.tile([128, 4, T], BF16, name=f"ct_{l}_{ti}", tag="ct")
                s2_n = ew.tile([128, 4, T], BF16, name=f"s2_{l}_{ti}", tag="s2")
                gm_n = (acts.tile([128, 4, 3, T], F8, name=f"gm_{l}_{ti}", tag="gm")
                        if l < 4 else None)
                m1_n = (acts.tile([128, 4, T], F8, name=f"m1_{l}_{ti}", tag="m1")
                        if l == 4 else None)
                q8_n = acts.tile([128, 4, T], F8, name=f"q8_{l}_{ti}", tag="q8")
                sqt = ew.tile([128, 4, 2, T], BF16, name=f"sq_{l}_{ti}", tag="sq")
                r2t = ew.tile([128, 4, T], BF16, name=f"r2_{l}_{ti}", tag="r2")
                for m in range(4):
                    pz = pp.tile([128, T], FP32, name=f"pz_{l}_{ti}_{m}", tag="pz", bufs=1)
                    pxyt = pp.tile([128, 3, T], FP32, name=f"pxyt_{l}_{ti}_{m}", tag="pxyt")
                    pxy = pxyt[:, 0:2, :]
                    pt = pxyt[:, 2, :]
                    msl = slice(m * 128, (m + 1) * 128)
                    if l == 1:
                        for j in range(2):
                            st, sp = j == 0, j == 1
                            dr(pxy[:, 0, :], wf_sb["x"], j, msl, c08[:, 2*j:2*j+2, :], st, sp)
                            dr(pxy[:, 1, :], wf_sb["y"], j, msl, c08[:, 2*j:2*j+2, :], st, sp)
                            dr(pz, w_sb[1], j, msl, v8[:, 2*j:2*j+2, :], st, sp)
                            dr(pt, wf_sb["q"], j, msl, v8[:, 2*j:2*j+2, :], st, sp)
                    else:
                        jp = m // 2
                        wl = w_sb[l]
                        dr(pxy[:, 0, :], wl, jp, msl, gm[:, 2*jp:2*jp+2, 0, :], True, True)
                        dr(pxy[:, 1, :], wl, jp, msl, gm[:, 2*jp:2*jp+2, 1, :], True, True)
                        for kk in range(2):
                            k = 2 * jp + kk
                            nc.tensor.matmul(pz, w16_sb[l][:, k, msl], v8[:, k, :],
                                             start=(kk == 0), stop=(kk == 1))
                        dr(pt, wl, jp, msl, gm[:, 2*jp:2*jp+2, 2, :], True, False)
                        dr(pt, wl, jp, msl, q8[:, 2*jp:2*jp+2, :], False, True)

                    # elementwise (v first: ct chain + q depend on it)
                    nc.scalar.activation(v8_n[:, m, :], pz, AF.Sin,
                                         bias=bias_sb[:, l, m, 0:1])
                    if CT_ENG[l] == "A":
                        nc.scalar.activation(ct_n[:, m, :], pz, AF.Sin,
                                             bias=bias_sb[:, l, m, 1:2])
                    else:
                        nc.vector.tensor_tensor(s2_n[:, m, :], v8_n[:, m, :],
                                                v8_n[:, m, :], ALU.mult)
                        nc.vector.tensor_scalar(ct_n[:, m, :], s2_n[:, m, :],
                                                -0.5, 1.0, op0=ALU.mult, op1=ALU.add)
                    if l < 4:
                        ct_b = _bcast(ct_n[:, m, :], 3)
                        nc.vector.tensor_tensor(gm_n[:, m, :, :], pxyt,
                                                ct_b, ALU.mult)
                    else:
                        nc.vector.tensor_tensor(m1_n[:, m, :], pt,
                                                ct_n[:, m, :], ALU.mult)
                    if SQ_ENG[l] == "A":
                        nc.scalar.activation(sqt[:, m, :, :], pxy, AF.Square)
                    elif SQ_ENG[l] == "D":
                        nc.vector.tensor_tensor(sqt[:, m, :, :], pxy,
                                                pxy, ALU.mult)
                    else:
                        nc.gpsimd.tensor_tensor(sqt[:, m, :, :], pxy,
                                                pxy, ALU.mult)
                    if R2_ENG[l] == "D":
                        nc.vector.tensor_tensor(r2t[:, m, :], sqt[:, m, 0, :],
                                                sqt[:, m, 1, :], ALU.add)
                    else:
                        nc.gpsimd.tensor_tensor(r2t[:, m, :], sqt[:, m, 0, :],
                                                sqt[:, m, 1, :], ALU.add)
                    if Q_ENG[l] == "P":
                        nc.gpsimd.tensor_tensor(q8_n[:, m, :], r2t[:, m, :],
                                                v8_n[:, m, :], ALU.mult)
                    else:
                        nc.vector.tensor_tensor(q8_n[:, m, :], r2t[:, m, :],
                                                v8_n[:, m, :], ALU.mult)
                v8, ct, q8 = v8_n, ct_n, q8_n
                if gm_n is not None:
                    gm = gm_n
                if m1_n is not None:
                    m1 = m1_n
                yield

            # final layer: E = -W5^T(m1+q) + k0^2 W5^T v + (f + k0^2 b5)
            pe_t = pp.tile([128, T], FP32, name=f"pe_{ti}", tag="pz", bufs=1)
            e = pe_t[0:1, :]
            idx = 0
            for s_, col in ((m1, 0), (q8, 0)):
                for j in range(2):
                    dr(e, w5_sb, j, slice(col, col + 1), s_[:, 2*j:2*j+2, :],
                       idx == 0, False)
                    idx += 1
            for k in range(4):
                nc.tensor.matmul(e, w516_sb[:, k, 1:2], v8[:, k, :],
                                 start=False, stop=False)
            nc.tensor.matmul(e, one_sb, fb_sb[0:1, csl], start=False, stop=True)
            scr = ew.tile([1, T], FP32, name=f"scr_{ti}", tag="scr", bufs=2)
            nc.scalar.activation(scr, e, AF.Square,
                                 accum_out=out_sb[0:1, ti : ti + 1])
            yield

        def boundary_chunks(ti):
            csl = slice(ti * T, (ti + 1) * T)
            vb = acts.tile([128, 4, T], F8, name=f"vb_0_{ti}", tag="vb")
            for m in range(4):
                pqb = pp.tile([128, T], FP32, name=f"bpq0_{ti}_{m}", tag="bz", bufs=1)
                nc.tensor.matmul(
                    pqb, w0_sb[:, m * 128 : (m + 1) * 128],
                    xb_sb[:, csl], start=True, stop=True,
                )
                nc.scalar.activation(vb[:, m, :], pqb,
                                     AF.Sin, bias=bias_sb[:, 0, m, 0:1])
                if m % 2:
                    yield
            for l in range(1, 5):
                vb_n = acts.tile([128, 4, T], F8, name=f"vb_{l}_{ti}", tag="vb")
                for m in range(4):
                    pqn = pp.tile([128, T], FP32, name=f"bpq_{l}_{ti}_{m}", tag="bz", bufs=1)
                    msl = slice(m * 128, (m + 1) * 128)
                    if l == 1:
                        for j in range(2):
                            dr(pqn, w_sb[1], j, msl,
                               vb[:, 2*j:2*j+2, :], j == 0, j == 1)
                    else:
                        jp = m // 2
                        dr(pqn, w_sb[l], jp, msl,
                           vb[:, 2*jp:2*jp+2, :], True, True)
                    nc.scalar.activation(vb_n[:, m, :], pqn,
                                         AF.Sin, bias=bias_sb[:, l, m, 0:1])
                    if m % 2:
                        yield
                vb = vb_n
            pe_t = pp.tile([128, T], FP32, name=f"bpe_{ti}", tag="bz", bufs=1)
            e = pe_t[0:1, :]
            for j in range(2):
                dr(e, w5_sb, j, slice(2, 3), vb[:, 2*j:2*j+2, :], j == 0, False)
            nc.tensor.matmul(e, one_sb, bb_sb[0:1, csl], start=False, stop=True)
            scr = ew.tile([1, T], FP32, name=f"bscr_{ti}", tag="scr", bufs=2)
            nc.scalar.activation(scr, e, AF.Square,
                                 accum_out=out_sb[0:1, 16 + ti : 17 + ti])
            yield

        import itertools
        bweave = itertools.chain(*[boundary_chunks(ti) for ti in range(ntb)])
        nchunks = ntb * 13
        done = 0
        for ti in range(ntd):
            for _ in domain_chunks(ti):
                pass
            want = (ti + 1) * nchunks // ntd
            while done < want and next(bweave, "END") != "END":
                done += 1
        for _ in bweave:
            pass

        nc.sync.dma_start(out=out_d[:], in_=out_sb)
    nc.compile()
    return nc


def _masks():
    layers = [2, 512, 256, 128, 64, 32, 1]
    width = [2, 512, 512, 512, 512, 512, 1]
    masks = {}
    for l in range(2, 5):
        nb_ = 2 ** (l - 1)
        bs1 = width[l] // nb_
        bs2 = 2 * layers[l + 1]
        m = np.zeros((512, 512), np.float32)
        for i in range(nb_):
            m[i * bs1 : (i + 1) * bs1, i * bs2 : (i + 1) * bs2] = 1.0
        masks[l] = m
    return masks


def _chunked(w):
    # [512, N] -> [128, 4, N] with out[p, kt, j] = w[kt*128 + p, j]
    n = w.shape[1]
    return np.ascontiguousarray(w.reshape(4, 128, n).transpose(1, 0, 2))


def host_prep(inputs, ntd=NTD, ntb=NTB):
    X = np.asarray(inputs["X_train"], np.float32)
    W = [np.asarray(inputs[f"W{i}"], np.float32) for i in range(6)]
    b = [np.asarray(inputs[f"b{i}"], np.float32) for i in range(6)]
    for l, m in _masks().items():
        W[l] = W[l] * m

    shared = {"w0": W[0].astype(bf16)}
    for l in (1, 2, 3, 4):
        shared[f"w{l}"] = _chunked(W[l]).astype(f8)
        shared[f"w16_{l}"] = _chunked(W[l]).astype(bf16)
    w5cat = np.concatenate(
        [-W[5], K0SQ * W[5], W[5], np.zeros((512, 13), np.float32)], axis=1)
    shared["w5"] = _chunked(w5cat).astype(f8)
    shared["w516"] = _chunked(w5cat[:, :3]).astype(bf16)

    bmat = np.stack([b[i][0] for i in range(5)], axis=0)  # [5, 512]
    bias = np.stack([bmat, bmat + PI_2], axis=-1)  # [5, 512, 2]
    # -> [128, 5, 4, 2]: bias_sb[p, l, m, j] = bias[l, m*128+p, j]
    shared["bias"] = np.ascontiguousarray(
        bias.reshape(5, 4, 128, 2).transpose(2, 0, 1, 3)
    ).astype(np.float32)

    zx0 = 2.0 * W[0][0, :]
    zy0 = 2.0 * W[0][1, :]
    c2 = zx0 ** 2 + zy0 ** 2
    shared["w1x"] = _chunked(zx0[:, None] * W[1]).astype(f8)
    shared["w1y"] = _chunked(zy0[:, None] * W[1]).astype(f8)
    shared["w1q"] = _chunked(c2[:, None] * W[1]).astype(f8)

    b5 = float(b[5][0, 0])
    td, tb = ntd * T, ntb * T
    per_core = []
    for c in range(NCORES):
        Xd = X[c * TDOM : c * TDOM + td]
        Xb = X[ND + c * TBND : ND + c * TBND + tb]
        xa = np.ascontiguousarray((2.0 * Xd - 1.0).T).astype(bf16)
        xbt = np.ascontiguousarray((2.0 * Xb - 1.0).T).astype(bf16)
        f = (K0SQ * np.sin(K0 * Xd[:, 0].astype(np.float64))
             * np.sin(K0 * Xd[:, 1].astype(np.float64)))
        fb = (f + K0SQ * b5).astype(bf16).reshape(1, td)
        bb = np.full((1, tb), b5, bf16)
        per_core.append({"xa": xa, "xb": xbt, "fb": fb, "bb": bb})
    return shared, per_core


_CACHE = {}


def _run(inputs, trace=False):
    key = "nc"
    if key not in _CACHE:
        _CACHE[key] = build_nc()
    nc = _CACHE[key]
    shared, per_core = host_prep(inputs)
    in_maps = [dict(shared, **pc) for pc in per_core]
    res = run_bass_kernel_spmd(nc, in_maps, core_ids=list(range(NCORES)), trace=trace)
    outs = [r["out"] for r in res.results]
    se = sum(float(o[0, :NTD].sum()) for o in outs)
    sb = sum(float(o[0, 16 : 16 + NTB].sum()) for o in outs)
    loss = se / ND + 100.0 * sb / NB
    return np.float32(loss), res


def kernel(**inputs):
    loss, _ = _run(inputs, trace=False)
    return np.asarray(loss)


# revision 39
# speedup vs baseline: 6.3417x; 6.3417x over previous
"""Trainium2 Bass kernel for the BsPINN Helmholtz loss (nn_BsPINN_45938970198305).

Forward-Laplacian propagation with fp8(e4m3) DoubleRow matmuls:
  streams per hidden layer l (all fp8 in SBUF, fp32 in PSUM):
    v8 = sin(z)                     gx8, gy8 = cos(z) * (zx, zy)
    m18 = cos(z) * zt               q8 = v * (zx^2 + zy^2)
  next layer (fp8 DoubleRow, K=256 per matmul):
    z' = W^T v8;  zx' = W^T gx8;  zy' = W^T gy8;  zt' = W^T m18 + W^T q8
  Layer-0 tangent constants folded into pre-scaled W1 copies (W1x/W1y/W1q);
  layer-0 emits sin/cos directly as fp8 (cos stays near 1; quantization of it
  is within the fp8 noise floor of the streams - verified numerically).
  Final: E = -W5^T(m1+q) + k0^2 W5^T v + (f + k0^2 b5), loss via E^2 accums.
  Boundary points: plain forward sin-pass, E_b = W5^T v + b5.

Sharding: data-parallel over points; 8 cores x (8192 domain + 2048 boundary);
weights replicated. Host combines 20 partial sums of squares per core.
"""

import numpy as np
import ml_dtypes

import concourse.bass as bass
import concourse.bacc as bacc_mod
import concourse.mybir as mybir
import concourse.tile as tile
from concourse.bass_utils import run_bass_kernel_spmd

bf16 = ml_dtypes.bfloat16
f8 = ml_dtypes.float8_e4m3
FP32 = mybir.dt.float32
BF16 = mybir.dt.bfloat16
F8 = mybir.dt.float8e4
AF = mybir.ActivationFunctionType
ALU = mybir.AluOpType
PM = mybir.MatmulPerfMode

NCORES = 8
ND, NB = 65536, 16384
TDOM, TBND = ND // NCORES, NB // NCORES  # 8192, 2048 points per core
T = 512                                  # points per tile
NTD, NTB = TDOM // T, TBND // T          # 16, 4
K0 = 8.0
K0SQ = K0 * K0
PI_2 = float(np.pi / 2)

# engine knobs (tuned against TimelineSim): where each elementwise op runs
SQ_ENG = {1: "A", 2: "D", 3: "P", 4: "A"}   # Square(pxy) per layer
Q_ENG = {1: "P", 2: "P", 3: "P", 4: "P"}    # q = v*r2 per layer
R2_ENG = {1: "D", 2: "D", 3: "D", 4: "D"}   # r2 = sqx+sqy per layer


def _bcast(tile_ap, n):
    """AP reading tile_ap (a [128, T] slice) broadcast n times on a middle dim."""
    return bass.AP(tile_ap.tensor, tile_ap.offset,
                   [tile_ap.ap[0], [0, n], tile_ap.ap[1]])


def build_nc(ntd=NTD, ntb=NTB):
    from contextlib import ExitStack

    td, tb = ntd * T, ntb * T
    nc = bacc_mod.Bacc("TRN2", target_bir_lowering=False)

    xa_d = nc.dram_tensor("xa", [2, td], BF16, kind="ExternalInput")
    xb_d = nc.dram_tensor("xb", [2, tb], BF16, kind="ExternalInput")
    fb_d = nc.dram_tensor("fb", [1, td], BF16, kind="ExternalInput")
    bb_d = nc.dram_tensor("bb", [1, tb], BF16, kind="ExternalInput")
    w0_d = nc.dram_tensor("w0", [2, 512], BF16, kind="ExternalInput")
    w_d = {
        l: nc.dram_tensor(f"w{l}", [128, 4, 512], F8, kind="ExternalInput")
        for l in (1, 2, 3, 4)
    }
    wf_d = {
        s: nc.dram_tensor(f"w1{s}", [128, 4, 512], F8, kind="ExternalInput")
        for s in ("x", "y", "q")
    }
    w5_d = nc.dram_tensor("w5", [128, 4, 16], F8, kind="ExternalInput")
    w16_d = {
        l: nc.dram_tensor(f"w16_{l}", [128, 4, 512], BF16, kind="ExternalInput")
        for l in (1, 2, 3, 4)
    }
    w516_d = nc.dram_tensor("w516", [128, 4, 3], BF16, kind="ExternalInput")
    bias_d = nc.dram_tensor("bias", [128, 5, 4, 2], FP32, kind="ExternalInput")
    out_d = nc.dram_tensor("out", [1, 32], FP32, kind="ExternalOutput")

    with tile.TileContext(nc) as tc, ExitStack() as ctx:
        singles = ctx.enter_context(tc.tile_pool(name="singles", bufs=1))
        acts = ctx.enter_context(tc.tile_pool(name="acts", bufs=3))
        ew = ctx.enter_context(tc.tile_pool(name="ew", bufs=3))
        pp = ctx.enter_context(tc.tile_pool(name="pp", bufs=2, space="PSUM"))

        # DMA order matters at startup: tile 0 needs bias (ACT warmup), w0,
        # xa chunk 0, then the L1 weight family; later layers' weights follow.
        bias_sb = singles.tile([128, 5, 4, 2], FP32, name="bias_sb")
        nc.sync.dma_start(out=bias_sb, in_=bias_d[:])
        w0_sb = singles.tile([2, 512], BF16, name="w0_sb")
        nc.sync.dma_start(out=w0_sb, in_=w0_d[:])
        xa_sb = singles.tile([2, td], BF16, name="xa_sb")
        fb_sb = singles.tile([1, td], BF16, name="fb_sb")
        s4 = slice(0, td // 4)
        nc.sync.dma_start(out=xa_sb[:, s4], in_=xa_d[:, s4])
        wf_sb = {}
        for s in ("x", "y", "q"):
            wf_sb[s] = singles.tile([128, 4, 512], F8, name=f"w1{s}_sb", tag=f"w1{s}_sb")
            nc.sync.dma_start(out=wf_sb[s], in_=wf_d[s][:])
        w_sb = {}
        w16_sb = {}
        for l in (1, 2, 3, 4):
            w_sb[l] = singles.tile([128, 4, 512], F8, name=f"w{l}_sb", tag=f"w{l}_sb")
            w16_sb[l] = singles.tile([128, 4, 512], BF16, name=f"w16_{l}_sb", tag=f"w16_{l}_sb")
        for l in (1, 2, 3, 4):
            nc.sync.dma_start(out=w_sb[l], in_=w_d[l][:])
            nc.sync.dma_start(out=w16_sb[l], in_=w16_d[l][:])
        for c4 in range(1, 4):
            s4 = slice(c4 * td // 4, (c4 + 1) * td // 4)
            nc.sync.dma_start(out=xa_sb[:, s4], in_=xa_d[:, s4])
        for c4 in range(4):
            s4 = slice(c4 * td // 4, (c4 + 1) * td // 4)
            nc.sync.dma_start(out=fb_sb[:, s4], in_=fb_d[:, s4])
        xb_sb = singles.tile([2, tb], BF16, name="xb_sb")
        nc.sync.dma_start(out=xb_sb, in_=xb_d[:])
        bb_sb = singles.tile([1, tb], BF16, name="bb_sb")
        nc.sync.dma_start(out=bb_sb, in_=bb_d[:])
        w5_sb = singles.tile([128, 4, 16], F8, name="w5_sb")
        nc.sync.dma_start(out=w5_sb, in_=w5_d[:])
        w516_sb = singles.tile([128, 4, 3], BF16, name="w516_sb")
        nc.sync.dma_start(out=w516_sb, in_=w516_d[:])

        out_sb = singles.tile([1, 32], FP32, name="out_sb")
        nc.vector.memset(out_sb, 0.0)
        one_sb = singles.tile([1, 1], BF16, name="one_sb")
        nc.vector.memset(one_sb, 1.0)

        # Warmup activation: absorbs the one-time ACT table load and bias-DMA
        # wait so later ACTIVATEs carry at most 2 sync waits.
        warm_sb = singles.tile([1, 1], FP32, name="warm_sb")
        nc.scalar.activation(warm_sb, bias_sb[0:1, 0, 0, 0:1], AF.Sin)

        def dr(out, wtile, jpair, msl_or_col, rhs, start, stop):
            nc.tensor.matmul(
                out, wtile[:, 2 * jpair : 2 * jpair + 2, msl_or_col], rhs,
                start=start, stop=stop, perf_mode=PM.DoubleRow,
            )

        # ---------------- tiles (boundary interleaved every 4th) ----------------
        def domain_chunks(ti):
            csl = slice(ti * T, (ti + 1) * T)

            # layer 0: z0 = W0^T xa (K=2, bf16); sin/cos pairs -> fp8
            v8 = acts.tile([128, 4, T], F8, name=f"v80_{ti}", tag="v80")
            c08 = acts.tile([128, 4, T], F8, name=f"c08_{ti}", tag="c08")
            for p in range(2):
                pq0 = pp.tile([128, 3, T], FP32, name=f"pq0_{ti}_{p}", tag="pxyt")
                for mm_ in range(2):
                    nc.tensor.matmul(
                        pq0[:, mm_, :],
                        w0_sb[:, (2 * p + mm_) * 128 : (2 * p + mm_ + 1) * 128],
                        xa_sb[:, csl], start=True, stop=True,
                    )
                nc.scalar.activation(v8[:, 2 * p : 2 * p + 2, :], pq0[:, 0:2, :],
                                     AF.Sin, bias=bias_sb[:, 0, 2 * p, 0:1])
                if C0_ENG == "A":
                    nc.scalar.activation(c08[:, 2 * p : 2 * p + 2, :], pq0[:, 0:2, :],
                                         AF.Sin, bias=bias_sb[:, 0, 2 * p, 1:2])
                else:
                    s08 = ew.tile([128, 2, T], BF16, name=f"s08_{ti}_{p}",
                                  tag="s08", bufs=2)
                    nc.vector.tensor_tensor(s08, v8[:, 2 * p : 2 * p + 2, :],
                                            v8[:, 2 * p : 2 * p + 2, :], ALU.mult)
                    nc.vector.tensor_scalar(c08[:, 2 * p : 2 * p + 2, :], s08,
                                            -0.5, 1.0, op0=ALU.mult, op1=ALU.add)
            yield

            # hidden layers 1..4: psum per m = [z | x | y | t]
            gm = None
            for l in range(1, 5):
                v8_n = acts.tile([128, 4, T], BF16, name=f"v8_{l}_{ti}", tag="v8")
                ct_n = acts.tile([128, 4, T], BF16, name=f"ct_{l}_{ti}", tag="ct")
                s2_n = ew.tile([128, 4, T], BF16, name=f"s2_{l}_{ti}", tag="s2")
                gm_n = (acts.tile([128, 4, 3, T], F8, name=f"gm_{l}_{ti}", tag="gm")
                        if l < 4 else None)
                m1_n = (acts.tile([128, 4, T], F8, name=f"m1_{l}_{ti}", tag="m1")
                        if l == 4 else None)
                q8_n = acts.tile([128, 4, T], F8, name=f"q8_{l}_{ti}", tag="q8")
                sqt = ew.tile([128, 4, 2, T], BF16, name=f"sq_{l}_{ti}", tag="sq")
                r2t = ew.tile([128, 4, T], BF16, name=f"r2_{l}_{ti}", tag="r2")
                for m in range(4):
                    pz = pp.tile([128, T], FP32, name=f"pz_{l}_{ti}_{m}", tag="pz", bufs=1)
                    pxyt = pp.tile([128, 3, T], FP32, name=f"pxyt_{l}_{ti}_{m}", tag="pxyt")
                    pxy = pxyt[:, 0:2, :]
                    pt = pxyt[:, 2, :]
                    msl = slice(m * 128, (m + 1) * 128)
                    if l == 1:
                        for j in range(2):
                            st, sp = j == 0, j == 1
                            dr(pxy[:, 0, :], wf_sb["x"], j, msl, c08[:, 2*j:2*j+2, :], st, sp)
                            dr(pxy[:, 1, :], wf_sb["y"], j, msl, c08[:, 2*j:2*j+2, :], st, sp)
                            dr(pz, w_sb[1], j, msl, v8[:, 2*j:2*j+2, :], st, sp)
                            dr(pt, wf_sb["q"], j, msl, v8[:, 2*j:2*j+2, :], st, sp)
                    else:
                        jp = m // 2
                        wl = w_sb[l]
                        dr(pxy[:, 0, :], wl, jp, msl, gm[:, 2*jp:2*jp+2, 0, :], True, True)
                        dr(pxy[:, 1, :], wl, jp, msl, gm[:, 2*jp:2*jp+2, 1, :], True, True)
                        for kk in range(2):
                            k = 2 * jp + kk
                            nc.tensor.matmul(pz, w16_sb[l][:, k, msl], v8[:, k, :],
                                             start=(kk == 0), stop=(kk == 1))
                        dr(pt, wl, jp, msl, gm[:, 2*jp:2*jp+2, 2, :], True, False)
                        dr(pt, wl, jp, msl, q8[:, 2*jp:2*jp+2, :], False, True)

                    # elementwise (v first: ct chain + q depend on it)
                    nc.scalar.activation(v8_n[:, m, :], pz, AF.Sin,
                                         bias=bias_sb[:, l, m, 0:1])
                    if CT_ENG[l] == "A":
                        nc.scalar.activation(ct_n[:, m, :], pz, AF.Sin,
                                             bias=bias_sb[:, l, m, 1:2])
                    else:
                        nc.vector.tensor_tensor(s2_n[:, m, :], v8_n[:, m, :],
                                                v8_n[:, m, :], ALU.mult)
                        nc.vector.tensor_scalar(ct_n[:, m, :], s2_n[:, m, :],
                                                -0.5, 1.0, op0=ALU.mult, op1=ALU.add)
                    if l < 4:
                        ct_b = _bcast(ct_n[:, m, :], 3)
                        nc.vector.tensor_tensor(gm_n[:, m, :, :], pxyt,
                                                ct_b, ALU.mult)
                    else:
                        nc.vector.tensor_tensor(m1_n[:, m, :], pt,
                                                ct_n[:, m, :], ALU.mult)
                    if SQ_ENG[l] == "A":
                        nc.scalar.activation(sqt[:, m, :, :], pxy, AF.Square)
                    elif SQ_ENG[l] == "D":
                        nc.vector.tensor_tensor(sqt[:, m, :, :], pxy,
                                                pxy, ALU.mult)
                    else:
                        nc.gpsimd.tensor_tensor(sqt[:, m, :, :], pxy,
                                                pxy, ALU.mult)
                    if R2_ENG[l] == "D":
                        nc.vector.tensor_tensor(r2t[:, m, :], sqt[:, m, 0, :],
                                                sqt[:, m, 1, :], ALU.add)
                    else:
                        nc.gpsimd.tensor_tensor(r2t[:, m, :], sqt[:, m, 0, :],
                                                sqt[:, m, 1, :], ALU.add)
                    if Q_ENG[l] == "P":
                        nc.gpsimd.tensor_tensor(q8_n[:, m, :], r2t[:, m, :],
                                                v8_n[:, m, :], ALU.mult)
                    else:
                        nc.vector.tensor_tensor(q8_n[:, m, :], r2t[:, m, :],
                                                v8_n[:, m, :], ALU.mult)
                v8, ct, q8 = v8_n, ct_n, q8_n
                if gm_n is not None:
                    gm = gm_n
                if m1_n is not None:
                    m1 = m1_n
                yield

            # final layer: E = -W5^T(m1+q) + k0^2 W5^T v + (f + k0^2 b5)
            pe_t = pp.tile([128, T], FP32, name=f"pe_{ti}", tag="pz", bufs=1)
            e = pe_t[0:1, :]
            idx = 0
            for s_, col in ((m1, 0), (q8, 0)):
                for j in range(2):
                    dr(e, w5_sb, j, slice(col, col + 1), s_[:, 2*j:2*j+2, :],
                       idx == 0, False)
                    idx += 1
            for k in range(4):
                nc.tensor.matmul(e, w516_sb[:, k, 1:2], v8[:, k, :],
                                 start=False, stop=False)
            nc.tensor.matmul(e, one_sb, fb_sb[0:1, csl], start=False, stop=True)
            scr = ew.tile([1, T], FP32, name=f"scr_{ti}", tag="scr", bufs=2)
            nc.scalar.activation(scr, e, AF.Square,
                                 accum_out=out_sb[0:1, ti : ti + 1])
            yield

        def boundary_chunks(ti):
            csl = slice(ti * T, (ti + 1) * T)
            vb = acts.tile([128, 4, T], F8, name=f"vb_0_{ti}", tag="vb")
            for m in range(4):
                pqb = pp.tile([128, T], FP32, name=f"bpq0_{ti}_{m}", tag="bz", bufs=1)
                nc.tensor.matmul(
                    pqb, w0_sb[:, m * 128 : (m + 1) * 128],
                    xb_sb[:, csl], start=True, stop=True,
                )
                nc.scalar.activation(vb[:, m, :], pqb,
                                     AF.Sin, bias=bias_sb[:, 0, m, 0:1])
                if m % 2:
                    yield
            for l in range(1, 5):
                vb_n = acts.tile([128, 4, T], F8, name=f"vb_{l}_{ti}", tag="vb")
                for m in range(4):
                    pqn = pp.tile([128, T], FP32, name=f"bpq_{l}_{ti}_{m}", tag="bz", bufs=1)
                    msl = slice(m * 128, (m + 1) * 128)
                    if l == 1:
                        for j in range(2):
                            dr(pqn, w_sb[1], j, msl,
                               vb[:, 2*j:2*j+2, :], j == 0, j == 1)
                    else:
                        jp = m // 2
                        dr(pqn, w_sb[l], jp, msl,
                           vb[:, 2*jp:2*jp+2, :], True, True)
                    nc.scalar.activation(vb_n[:, m, :], pqn,
                                         AF.Sin, bias=bias_sb[:, l, m, 0:1])
                    if m % 2:
                        yield
                vb = vb_n
            pe_t = pp.tile([128, T], FP32, name=f"bpe_{ti}", tag="bz", bufs=1)
            e = pe_t[0:1, :]
            for j in range(2):
                dr(e, w5_sb, j, slice(2, 3), vb[:, 2*j:2*j+2, :], j == 0, False)
            nc.tensor.matmul(e, one_sb, bb_sb[0:1, csl], start=False, stop=True)
            scr = ew.tile([1, T], FP32, name=f"bscr_{ti}", tag="scr", bufs=2)
            nc.scalar.activation(scr, e, AF.Square,
                                 accum_out=out_sb[0:1, 16 + ti : 17 + ti])
            yield

        import itertools
        bweave = itertools.chain(*[boundary_chunks(ti) for ti in range(ntb)])
        nchunks = ntb * 13
        done = 0
        for ti in range(ntd):
            for _ in domain_chunks(ti):
                pass
            want = (ti + 1) * nchunks // ntd
            while done < want and next(bweave, "END") != "END":
                done += 1
        for _ in bweave:
            pass

        nc.sync.dma_start(out=out_d[:], in_=out_sb)
    nc.compile()
    return nc


def _masks():
    layers = [2, 512, 256, 128, 64, 32, 1]
    width = [2, 512, 512, 512, 512, 512, 1]
    masks = {}
    for l in range(2, 5):
        nb_ = 2 ** (l - 1)
        bs1 = width[l] // nb_
        bs2 = 2 * layers[l + 1]
        m = np.zeros((512, 512), np.float32)
        for i in range(nb_):
            m[i * bs1 : (i + 1) * bs1, i * bs2 : (i + 1) * bs2] = 1.0
        masks[l] = m
    return masks


def _chunked(w):
    # [512, N] -> [128, 4, N] with out[p, kt, j] = w[kt*128 + p, j]
    n = w.shape[1]
    return np.ascontiguousarray(w.reshape(4, 128, n).transpose(1, 0, 2))


def host_prep(inputs, ntd=NTD, ntb=NTB):
    X = np.asarray(inputs["X_train"], np.float32)
    W = [np.asarray(inputs[f"W{i}"], np.float32) for i in range(6)]
    b = [np.asarray(inputs[f"b{i}"], np.float32) for i in range(6)]
    for l, m in _masks().items():
        W[l] = W[l] * m

    shared = {"w0": W[0].astype(bf16)}
    for l in (1, 2, 3, 4):
        shared[f"w{l}"] = _chunked(W[l]).astype(f8)
        shared[f"w16_{l}"] = _chunked(W[l]).astype(bf16)
    w5cat = np.concatenate(
        [-W[5], K0SQ * W[5], W[5], np.zeros((512, 13), np.float32)], axis=1)
    shared["w5"] = _chunked(w5cat).astype(f8)
    shared["w516"] = _chunked(w5cat[:, :3]).astype(bf16)

    bmat = np.stack([b[i][0] for i in range(5)], axis=0)  # [5, 512]
    bias = np.stack([bmat, bmat + PI_2], axis=-1)  # [5, 512, 2]
    # -> [128, 5, 4, 2]: bias_sb[p, l, m, j] = bias[l, m*128+p, j]
    shared["bias"] = np.ascontiguousarray(
        bias.reshape(5, 4, 128, 2).transpose(2, 0, 1, 3)
    ).astype(np.float32)

    zx0 = 2.0 * W[0][0, :]
    zy0 = 2.0 * W[0][1, :]
    c2 = zx0 ** 2 + zy0 ** 2
    shared["w1x"] = _chunked(zx0[:, None] * W[1]).astype(f8)
    shared["w1y"] = _chunked(zy0[:, None] * W[1]).astype(f8)
    shared["w1q"] = _chunked(c2[:, None] * W[1]).astype(f8)

    b5 = float(b[5][0, 0])
    td, tb = ntd * T, ntb * T
    per_core = []
    for c in range(NCORES):
        Xd = X[c * TDOM : c * TDOM + td]
        Xb = X[ND + c * TBND : ND + c * TBND + tb]
        xa = np.ascontiguousarray((2.0 * Xd - 1.0).T).astype(bf16)
        xbt = np.ascontiguousarray((2.0 * Xb - 1.0).T).astype(bf16)
        f = (K0SQ * np.sin(K0 * Xd[:, 0].astype(np.float64))
             * np.sin(K0 * Xd[:, 1].astype(np.float64)))
        fb = (f + K0SQ * b5).astype(bf16).reshape(1, td)
        bb = np.full((1, tb), b5, bf16)
        per_core.append({"xa": xa, "xb": xbt, "fb": fb, "bb": bb})
    return shared, per_core


_CACHE = {}


def _run(inputs, trace=False):
    key = "nc"
    if key not in _CACHE:
        _CACHE[key] = build_nc()
    nc = _CACHE[key]
    shared, per_core = host_prep(inputs)
    in_maps = [dict(shared, **pc) for pc in per_core]
    res = run_bass_kernel_spmd(nc, in_maps, core_ids=list(range(NCORES)), trace=trace)
    outs = [r["out"] for r in res.results]
    se = sum(float(o[0, :NTD].sum()) for o in outs)
    sb = sum(float(o[0, 16 : 16 + NTB].sum()) for o in outs)
    loss = se / ND + 100.0 * sb / NB
    return np.float32(loss), res


def kernel(**inputs):
    loss, _ = _run(inputs, trace=False)
    return np.asarray(loss)
